# revision 8
# baseline (speedup 1.0000x reference)
"""Masked multi-head buffer attention on 8 TRN2 NeuronCores.

Problem shapes: x (2, 2048, 1024), buffer (2, 2048, 1024), mask (2, 2048, 2048),
Wq/Wk/Wv (1024, 1024), biases (1024,). Output (2, 2048, 1024) fp32.

Sharding: core c in 0..7 handles batch b = c//4 and head group g = c%4
(4 heads of 16). Pure data/head parallelism -- no collectives.

Host prep (free, not on HW critical path): transpose x/buffer/W/mask, fold the
bias into an extra contraction row, append a ones output-column per head to V
(gives softmax row-sums via the AV matmul), pack x, buffer and mask into
q-/k-column quarters laid out so each quarter is one fully-contiguous DMA,
cast everything to bf16.

Device schedule (per core), engineered from perfetto profiles:
  warmup: dummy matmuls from t=0 ramp the PE p-state (the clock only reaches
      2.4GHz after ~3us of continuous execution).
  DMA priority order (sync queue is FIFO): wk, b quarter 0, wv, wq, mask
      tiles 0-3, x quarter 0, then b quarters / mask groups / x quarters in
      first-use order. Everything the first attention iterations touch lands
      in the first ~17us of wire time.
  projections are uniform "units" (8 chunk-matmuls into one PSUM bank + a
      copy): kT/v units are quarter-local so they become ready as each b
      quarter lands. A few units run before the attention loop; the rest
      drip into it as micro-ops against static (ready, deadline) windows.
  attention (per (hp, qb) block, per k-tile of 128):
      ST[k, q] one tile ahead on PE (two heads packed in row groups 0-63 /
      64-127, which the HW co-issues), exp on ScalarE (~1.0us/iter), mask
      multiply on VectorE, OT += v_ext^T P accumulated in PSUM.
  epilogue per (head, q-block): PE-transpose OT, strided reciprocal of the
      sum column, per-chunk scale into a packed tile, one DMA per head.
"""

import os
import sys

import numpy as np

for _p in ("/opt/trn_rl_repo", "/root/.axon_site/_ro/trn_rl_repo"):
    if os.path.isdir(_p) and _p not in sys.path:
        sys.path.insert(0, _p)

import ml_dtypes

B, Q, KS, D = 2, 2048, 2048, 1024
H, DK, DV = 16, 64, 64
HPC = 4  # heads per core
NCORES = 8
NDC = 9  # contraction chunks of 128 (incl. bias/ones row block)
DPAD = NDC * 128  # 1152
VW = DV + 1  # per-head v width incl. ones column
QB = 512  # q block in attention phase
KT = KS // 128
NQB = Q // QB
SCALE = 1.0 / np.sqrt(DK)
NWARM = 30  # PE warmup dummy matmuls

_GRAPH = None


def _build_graph(skip_qk_bias=False):
    import concourse.bass as bass
    import concourse.mybir as mybir
    import concourse.tile as tile
    from concourse import bacc
    from concourse.bass import ds, ts
    from concourse.masks import make_identity

    f32 = mybir.dt.float32
    bf16 = mybir.dt.bfloat16
    EXP = mybir.ActivationFunctionType.Exp
    NQK = NDC - 1 if skip_qk_bias else NDC  # contraction chunks for q/k

    nc = bacc.Bacc(None)
    xq_d = nc.declare_dram_parameter("xq", [NQB, 128, NDC * QB], bf16, isOutput=False)
    bq_d = nc.declare_dram_parameter("bq", [NQB, 128, NDC * QB], bf16, isOutput=False)
    wq = nc.declare_dram_parameter("wq", [DPAD, HPC * DK], bf16, isOutput=False)
    wk = nc.declare_dram_parameter("wk", [DPAD, HPC * DK], bf16, isOutput=False)
    wv = nc.declare_dram_parameter("wv", [DPAD, HPC * VW], bf16, isOutput=False)
    mq_d = nc.declare_dram_parameter("mq", [NQB, 128, KT * QB], bf16, isOutput=False)
    out = nc.declare_dram_parameter("out", [Q, HPC * DV], f32, isOutput=True)

    with tile.TileContext(nc) as tc:
        with (
            tc.tile_pool(name="weights", bufs=1) as wpool,
            tc.tile_pool(name="bigin", bufs=1) as xpool,
            tc.tile_pool(name="maskp", bufs=1) as mpool,
            tc.tile_pool(name="qkv", bufs=1) as qkvpool,
            tc.tile_pool(name="consts", bufs=1) as cpool,
        ):
            ident = cpool.tile([128, 128], bf16, tag="ident")
            make_identity(nc, ident[:])
            warm = cpool.tile([128, QB], bf16, tag="warm")
            nc.gpsimd.memset(warm[:], 0.0)

            # persistent qkv SBUF
            qT_sb = [
                qkvpool.tile([128, Q], bf16, tag=f"qT{i}", name=f"qT{i}")
                for i in range(2)
            ]
            kT_sb = [
                qkvpool.tile([128, KS], bf16, tag=f"kT{i}", name=f"kT{i}")
                for i in range(2)
            ]
            v_sb = [
                qkvpool.tile([128, HPC * VW], bf16, tag=f"v{i}", name=f"v{i}")
                for i in range(KT)
            ]
            if skip_qk_bias:
                # ones columns filled once at boot; v copies never touch them
                for it in range(KT):
                    nc.gpsimd.memset(v_sb[it][:, DV::VW], 1.0)

            # ---------------- SBUF destination tiles ----------------
            w_sb = {}
            xq_g = [
                xpool.tile([128, NQK * QB], bf16, tag=f"xq{i}", name=f"xq{i}")
                for i in range(NQB)
            ]
            bq_g = [
                xpool.tile([128, NQK * QB], bf16, tag=f"bq{i}", name=f"bq{i}")
                for i in range(NQB)
            ]
            mq_g = [
                mpool.tile([128, KT * QB], bf16, tag=f"mq{i}", name=f"mq{i}")
                for i in range(NQB)
            ]

            def w_dma(nm, dram, width):
                t = wpool.tile([128, NDC * width], bf16, tag=nm, name=nm)
                src = dram[:].rearrange("(c p) w -> p c w", p=128)
                nc.sync.dma_start(
                    out=t[:].rearrange("p (c w) -> p c w", c=NDC), in_=src
                )
                w_sb[nm] = t

            def wslice(nm, dc, lo, size):
                width = (HPC * DK) if nm in ("wq", "wk") else (HPC * VW)
                return w_sb[nm][:, ds(dc * width + lo, size)]

            def xsl(dc, qc, lo=0, size=QB):
                return xq_g[qc][:, ds(dc * QB + lo, size)]

            def bsl(dc, kq, lo=0, size=QB):
                return bq_g[kq][:, ds(dc * QB + lo, size)]

            def mslice(kt, qb):
                return mq_g[qb][:, ds(kt * QB, QB)]

            # ---------------- DMAs in priority order ----------------
            w_dma("wk", wk, HPC * DK)
            nc.sync.dma_start(out=bq_g[0][:], in_=bq_d[0, :, 0 : NQK * QB])
            w_dma("wv", wv, HPC * VW)
            w_dma("wq", wq, HPC * DK)
            # mask group 0 of quarter 0 (k-tiles 0..3)
            nc.sync.dma_start(
                out=mq_g[0][:, 0 : 4 * QB], in_=mq_d[0, :, 0 : 4 * QB]
            )
            nc.sync.dma_start(out=xq_g[0][:], in_=xq_d[0, :, 0 : NQK * QB])
            for grp in range(1, 4):
                nc.sync.dma_start(
                    out=bq_g[grp][:], in_=bq_d[grp, :, 0 : NQK * QB]
                )
                nc.sync.dma_start(
                    out=mq_g[0][:, ds(grp * 4 * QB, 4 * QB)],
                    in_=mq_d[0, :, ds(grp * 4 * QB, 4 * QB)],
                )
            for qb in range(1, NQB):
                nc.sync.dma_start(
                    out=xq_g[qb][:], in_=xq_d[qb, :, 0 : NQK * QB]
                )
                nc.sync.dma_start(out=mq_g[qb][:], in_=mq_d[qb])

            # ---------------- projection units ----------------
            # bufs=2 so unit N+1's matmuls overlap unit N's PSUM->SBUF copy
            pjl = tc.tile_pool(name="pjl", bufs=2, space="PSUM")
            pjlpool = pjl.__enter__()

            def warmup():
                ps = pjlpool.tile([128, QB], f32, tag="pjl", name="pjl")
                for _ in range(NWARM):
                    nc.tensor.matmul(
                        ps[:],
                        ident[:],
                        warm[:],
                        start=True,
                        stop=True,
                        skip_group_check=True,
                    )

            def v_unit(it):
                """v tile it (k-positions it*128..+128): 8 matmuls + copy."""
                ps = pjlpool.tile([128, QB], f32, tag="pjl", name="pjl")
                ops = []
                for dc in range(NQK):
                    ops.append(
                        lambda dc=dc, ps=ps: nc.tensor.matmul(
                            ps[:, : HPC * VW],
                            bsl(dc, it // 4, (it % 4) * 128, 128),
                            wslice("wv", dc, 0, HPC * VW),
                            start=(dc == 0),
                            stop=(dc == NQK - 1),
                        )
                    )

                def fin(ps=ps):
                    if skip_qk_bias:
                        nc.vector.tensor_copy(
                            v_sb[it][:]
                            .rearrange("p (h w) -> p h w", h=HPC)[:, :, 0:DV],
                            ps[:, : HPC * VW]
                            .rearrange("p (h w) -> p h w", h=HPC)[:, :, 0:DV],
                        )
                    else:
                        nc.vector.tensor_copy(v_sb[it][:], ps[:, : HPC * VW])

                ops.append(fin)
                return ops

            def q_unit(pair, qc, on_scalar=False):
                """qT_sb[pair][:, qc*512:+512]: 8 matmuls + copy."""
                ps = pjlpool.tile([128, QB], f32, tag="pjl", name="pjl")
                ops = []
                for dc in range(NQK):
                    ops.append(
                        lambda dc=dc, ps=ps: nc.tensor.matmul(
                            ps[:],
                            wslice("wq", dc, pair * 128, 128),
                            xsl(dc, qc),
                            start=(dc == 0),
                            stop=(dc == NQK - 1),
                        )
                    )
                dst = qT_sb[pair]
                if on_scalar:
                    ops.append(
                        lambda ps=ps: nc.scalar.copy(dst[:, ts(qc, QB)], ps[:])
                    )
                else:
                    ops.append(
                        lambda ps=ps: nc.vector.tensor_copy(
                            dst[:, ts(qc, QB)], ps[:]
                        )
                    )
                return ops

            def k_unit(pair, qc, on_scalar=False):
                """kT_sb[pair][:, qc*512:+512]: 8 matmuls + copy."""
                ps = pjlpool.tile([128, QB], f32, tag="pjl", name="pjl")
                ops = []
                for dc in range(NQK):
                    ops.append(
                        lambda dc=dc, ps=ps: nc.tensor.matmul(
                            ps[:],
                            wslice("wk", dc, pair * 128, 128),
                            bsl(dc, qc),
                            start=(dc == 0),
                            stop=(dc == NQK - 1),
                        )
                    )
                dst = kT_sb[pair]
                if on_scalar:
                    ops.append(
                        lambda ps=ps: nc.scalar.copy(dst[:, ts(qc, QB)], ps[:])
                    )
                else:
                    ops.append(
                        lambda ps=ps: nc.vector.tensor_copy(
                            dst[:, ts(qc, QB)], ps[:]
                        )
                    )
                return ops

            # pre-attention: warmup, then the units block 0 needs first
            warmup()
            for ops in (
                k_unit(0, 0, on_scalar=True),
                k_unit(1, 0, on_scalar=True),
                v_unit(0),
                v_unit(1),
                q_unit(0, 0, on_scalar=True),
                v_unit(2),
                v_unit(3),
            ):
                for op in ops:
                    op()

            # drip queue: (ready_iter, emit_by_iter, unit ops)
            units = []
            units.append((0, 2, k_unit(0, 1)))
            for it in range(4, 8):
                units.append((0, it - 1, v_unit(it)))
            units.append((2, 6, k_unit(0, 2)))
            for it in range(8, 12):
                units.append((3, it - 1, v_unit(it)))
            units.append((4, 10, k_unit(0, 3)))
            for it in range(12, KT):
                units.append((5, it - 1, v_unit(it)))
            units.append((7, 11, q_unit(0, 1)))
            units.append((12, 27, q_unit(0, 2)))
            units.append((17, 43, q_unit(0, 3)))
            units.append((18, 50, k_unit(1, 1)))
            units.append((19, 54, k_unit(1, 2)))
            units.append((20, 58, k_unit(1, 3)))
            units.append((21, 56, q_unit(1, 0)))
            units.append((22, 71, q_unit(1, 1)))
            units.append((23, 87, q_unit(1, 2)))
            units.append((24, 103, q_unit(1, 3)))
            units.sort(key=lambda u: u[1])
            wq_flat = []
            for rdy, eb, ops in units:
                for op in ops:
                    wq_flat.append((rdy, eb, op))
            wq_i = 0

            def drip(giter):
                nonlocal wq_i
                n = 0
                while wq_i < len(wq_flat):
                    rdy, eb, op = wq_flat[wq_i]
                    if eb > giter and (n >= 3 or rdy > giter):
                        break
                    op()
                    wq_i += 1
                    n += 1

            # ---------------- attention ----------------
            with (
                tc.tile_pool(name="stp", bufs=2, space="PSUM") as stp,
                tc.tile_pool(name="otp", bufs=1, space="PSUM") as otp,
                tc.tile_pool(name="ptp", bufs=8) as ptp,
                tc.tile_pool(name="epp", bufs=2) as epp,
                tc.tile_pool(name="osp", bufs=3) as osp,
            ):

                def do_st(hp, qb, kt):
                    st = stp.tile([128, 2 * QB], f32, tag="st", name="st")
                    nc.tensor.matmul(
                        st[:, 0:QB],
                        kT_sb[hp][0:64, ts(kt, 128)],
                        qT_sb[hp][0:64, ds(qb * QB, QB)],
                        start=True,
                        stop=True,
                    )
                    nc.tensor.matmul(
                        st[:, QB : 2 * QB],
                        kT_sb[hp][64:128, ts(kt, 128)],
                        qT_sb[hp][64:128, ds(qb * QB, QB)],
                        start=True,
                        stop=True,
                    )
                    return st

                blocks = [(hp, qb) for hp in range(2) for qb in range(NQB)]
                st_cur = do_st(blocks[0][0], blocks[0][1], 0)
                for bi, (hp, qb) in enumerate(blocks):
                    qlo = qb * QB
                    ot0 = otp.tile([128, QB], f32, tag="ot0", name="ot0")
                    ot1 = otp.tile([128, QB], f32, tag="ot1", name="ot1")
                    for kt in range(KT):
                        giter = bi * KT + kt
                        if kt + 1 < KT:
                            st_next = do_st(hp, qb, kt + 1)
                        elif bi + 1 < len(blocks):
                            nhp, nqb = blocks[bi + 1]
                            st_next = do_st(nhp, nqb, 0)
                        else:
                            st_next = None
                        pt = ptp.tile([128, 2 * QB], bf16, tag="pt", name="pt")
                        nc.scalar.activation(pt[:], st_cur[:], EXP, scale=SCALE)
                        msl = mslice(kt, qb)
                        mbc = bass.AP(
                            tensor=msl.tensor,
                            offset=msl.offset,
                            ap=[msl.ap[0], [0, 2], [1, QB]],
                        )
                        nc.vector.tensor_mul(pt[:], pt[:], mbc)
                        nc.tensor.matmul(
                            ot0[:VW, :],
                            v_sb[kt][:, ds((2 * hp) * VW, VW)],
                            pt[:, 0:QB],
                            start=(kt == 0),
                            stop=(kt == KT - 1),
                        )
                        nc.tensor.matmul(
                            ot1[:VW, :],
                            v_sb[kt][:, ds((2 * hp + 1) * VW, VW)],
                            pt[:, QB : 2 * QB],
                            start=(kt == 0),
                            stop=(kt == KT - 1),
                        )
                        drip(giter + 1)
                        st_cur = st_next
                    # epilogue for the two heads of this (hp, qb)
                    last_block = bi == len(blocks) - 1
                    for hh, ot_acc in ((2 * hp, ot0), (2 * hp + 1, ot1)):
                        ot_sbuf = epp.tile(
                            [128, QB], bf16, tag="otsb", name="otsb"
                        )
                        nc.vector.tensor_copy(ot_sbuf[:VW, :], ot_acc[:VW, :])
                        nqt = QB // 128
                        VWP = VW + 1  # pad stride so PSUM stays 4B-aligned
                        # transpose target shares the pjl pool's PSUM slots
                        tr = pjlpool.tile(
                            [128, nqt * VWP], bf16, tag="pjl", name="tr"
                        )
                        for qt in range(nqt):
                            nc.tensor.transpose(
                                tr[:, ds(qt * VWP, VW)],
                                ot_sbuf[:VW, ts(qt, 128)],
                                ident[:VW, :VW],
                            )
                        rec = epp.tile([128, nqt], f32, tag="rec", name="rec")
                        nc.vector.reciprocal(rec[:], tr[:, DV::VWP])
                        osb = osp.tile(
                            [128, nqt * DV], f32, tag="osb", name="osb"
                        )
                        for qt in range(nqt):
                            if last_block:
                                nc.scalar.activation(
                                    osb[:, ds(qt * DV, DV)],
                                    tr[:, ds(qt * VWP, DV)],
                                    mybir.ActivationFunctionType.Copy,
                                    scale=rec[:, qt : qt + 1],
                                )
                            else:
                                nc.vector.tensor_scalar_mul(
                                    osb[:, ds(qt * DV, DV)],
                                    tr[:, ds(qt * VWP, DV)],
                                    rec[:, qt : qt + 1],
                                )
                        dst = out[ds(qlo, QB), ds(hh * DV, DV)].rearrange(
                            "(t p) d -> p t d", p=128
                        )
                        nc.sync.dma_start(
                            out=dst,
                            in_=osb[:].rearrange("p (t d) -> p t d", t=nqt),
                        )
            pjl.__exit__(None, None, None)
    nc.compile()
    return nc


def _get_graph(skip_qk_bias=False):
    global _GRAPH
    if _GRAPH is None or _GRAPH[1] != skip_qk_bias:
        _GRAPH = (_build_graph(skip_qk_bias), skip_qk_bias)
    return _GRAPH[0]


def _pack_quarters(aT):
    """[DPAD, N] -> [NQB, 128, NDC*512]: quarter-major, contiguous DMAs."""
    return np.ascontiguousarray(
        aT.reshape(NDC, 128, NQB, QB)
        .transpose(2, 1, 0, 3)
        .reshape(NQB, 128, NDC * QB)
    )


def _prep_core_inputs(c, x, buffer, mask, Wq, bq, Wk, bk, Wv, bv):
    bf = ml_dtypes.bfloat16
    b, g = divmod(c, 4)
    hs = slice(g * HPC * DK, (g + 1) * HPC * DK)

    xTa = np.zeros((DPAD, Q), np.float32)
    xTa[:D] = x[b].T
    xTa[D] = 1.0
    bTa = np.zeros((DPAD, KS), np.float32)
    bTa[:D] = buffer[b].T
    bTa[D] = 1.0
    wqa = np.zeros((DPAD, HPC * DK), np.float32)
    wqa[:D] = Wq[hs].T
    wqa[D] = bq[hs]
    wka = np.zeros((DPAD, HPC * DK), np.float32)
    wka[:D] = Wk[hs].T
    wka[D] = bk[hs]
    wva = np.zeros((DPAD, HPC * VW), np.float32)
    for hh in range(HPC):
        gh = g * HPC + hh
        wva[:D, hh * VW : hh * VW + DV] = Wv[gh * DV : (gh + 1) * DV].T
        wva[D, hh * VW : hh * VW + DV] = bv[gh * DV : (gh + 1) * DV]
        wva[D, hh * VW + DV] = 1.0
    mTa = mask[b].T.astype(np.float32)  # [KS, Q]
    # mask quarters: mq[qb][p][kt*QB + w] = mTa[kt*128 + p][qb*QB + w]
    mqa = (
        mTa.reshape(KT, 128, NQB, QB)
        .transpose(2, 1, 0, 3)
        .reshape(NQB, 128, KT * QB)
    )
    return {
        "xq": _pack_quarters(xTa).astype(bf),
        "bq": _pack_quarters(bTa).astype(bf),
        "wq": wqa.astype(bf),
        "wk": wka.astype(bf),
        "wv": wva.astype(bf),
        "mq": np.ascontiguousarray(mqa).astype(bf),
    }


def kernel(**inputs):
    x = np.asarray(inputs["x"], dtype=np.float32)
    buffer = np.asarray(inputs["buffer"], dtype=np.float32)
    mask = np.asarray(inputs["mask"])
    Wq = np.asarray(inputs["Wq"], dtype=np.float32)
    bq = np.asarray(inputs["bq"], dtype=np.float32)
    Wk = np.asarray(inputs["Wk"], dtype=np.float32)
    bk = np.asarray(inputs["bk"], dtype=np.float32)
    Wv = np.asarray(inputs["Wv"], dtype=np.float32)
    bv = np.asarray(inputs["bv"], dtype=np.float32)

    from concourse.bass_utils import run_bass_kernel_spmd

    skip_qk_bias = not (bq.any() or bk.any())
    nc = _get_graph(skip_qk_bias)
    in_maps = [
        _prep_core_inputs(c, x, buffer, mask, Wq, bq, Wk, bk, Wv, bv)
        for c in range(NCORES)
    ]
    res = run_bass_kernel_spmd(nc, in_maps, core_ids=list(range(NCORES)))
    full = np.empty((B, Q, H * DV), np.float32)
    for c in range(NCORES):
        b, g = divmod(c, 4)
        full[b, :, g * HPC * DV : (g + 1) * HPC * DV] = res.results[c]["out"]
    return full


# revision 10
# speedup vs baseline: 1.0722x; 1.0722x over previous
"""Masked multi-head buffer attention on 8 TRN2 NeuronCores.

Problem shapes: x (2, 2048, 1024), buffer (2, 2048, 1024), mask (2, 2048, 2048),
Wq/Wk/Wv (1024, 1024), biases (1024,). Output (2, 2048, 1024) fp32.

Sharding: core c in 0..7 handles batch b = c//4 and head group g = c%4
(4 heads of 16). Pure data/head parallelism -- no collectives.

Host prep (free, not on HW critical path): transpose x/buffer/W/mask, fold the
bias into an extra contraction row, append a ones output-column per head to V
(gives softmax row-sums via the AV matmul), pack x, buffer and mask into
q-/k-column quarters laid out so each quarter is one fully-contiguous DMA,
cast everything to bf16.

Device schedule (per core), engineered from perfetto profiles:
  warmup: dummy matmuls from t=0 ramp the PE p-state (the clock only reaches
      2.4GHz after ~3us of continuous execution).
  DMA priority order (sync queue is FIFO): wk, b quarter 0, wv, wq, mask
      tiles 0-3, x quarter 0, then b quarters / mask groups / x quarters in
      first-use order. Everything the first attention iterations touch lands
      in the first ~17us of wire time.
  projections are uniform "units" (8 chunk-matmuls into one PSUM bank + a
      copy): kT/v units are quarter-local so they become ready as each b
      quarter lands. A few units run before the attention loop; the rest
      drip into it as micro-ops against static (ready, deadline) windows.
  attention (per (hp, qb) block, per k-tile of 128):
      ST[k, q] one tile ahead on PE (two heads packed in row groups 0-63 /
      64-127, which the HW co-issues), exp on ScalarE (~1.0us/iter), mask
      multiply on VectorE, OT += v_ext^T P accumulated in PSUM.
  epilogue per (head, q-block): PE-transpose OT, strided reciprocal of the
      sum column, per-chunk scale into a packed tile, one DMA per head.
"""

import os
import sys

import numpy as np

for _p in ("/opt/trn_rl_repo", "/root/.axon_site/_ro/trn_rl_repo"):
    if os.path.isdir(_p) and _p not in sys.path:
        sys.path.insert(0, _p)

import ml_dtypes

B, Q, KS, D = 2, 2048, 2048, 1024
H, DK, DV = 16, 64, 64
HPC = 4  # heads per core
NCORES = 8
NDC = 9  # contraction chunks of 128 (incl. bias/ones row block)
DPAD = NDC * 128  # 1152
VW = DV + 1  # per-head v width incl. ones column
QB = 512  # q block in attention phase
KT = KS // 128
NQB = Q // QB
SCALE = 1.0 / np.sqrt(DK)
NWARM = 30  # PE warmup dummy matmuls

_GRAPH = None


def _build_graph(skip_qk_bias=False):
    import concourse.bass as bass
    import concourse.mybir as mybir
    import concourse.tile as tile
    from concourse import bacc
    from concourse.bass import ds, ts
    from concourse.masks import make_identity

    f32 = mybir.dt.float32
    bf16 = mybir.dt.bfloat16
    EXP = mybir.ActivationFunctionType.Exp
    NQK = NDC - 1 if skip_qk_bias else NDC  # contraction chunks for q/k

    nc = bacc.Bacc(None)
    xq_d = nc.declare_dram_parameter("xq", [NQB, 128, NDC * QB], bf16, isOutput=False)
    bq_d = nc.declare_dram_parameter("bq", [NQB, 128, NDC * QB], bf16, isOutput=False)
    wq = nc.declare_dram_parameter("wq", [DPAD, HPC * DK], bf16, isOutput=False)
    wk = nc.declare_dram_parameter("wk", [DPAD, HPC * DK], bf16, isOutput=False)
    wv = nc.declare_dram_parameter("wv", [DPAD, HPC * VW], bf16, isOutput=False)
    mq_d = nc.declare_dram_parameter("mq", [NQB, 128, KT * QB], bf16, isOutput=False)
    out = nc.declare_dram_parameter("out", [Q, HPC * DV], f32, isOutput=True)

    with tile.TileContext(nc) as tc:
        with (
            tc.tile_pool(name="weights", bufs=1) as wpool,
            tc.tile_pool(name="bigin", bufs=1) as xpool,
            tc.tile_pool(name="maskp", bufs=1) as mpool,
            tc.tile_pool(name="qkv", bufs=1) as qkvpool,
            tc.tile_pool(name="consts", bufs=1) as cpool,
        ):
            ident = cpool.tile([128, 128], bf16, tag="ident")
            make_identity(nc, ident[:])
            warm = cpool.tile([128, QB], bf16, tag="warm")
            nc.gpsimd.memset(warm[:], 0.0)

            # persistent qkv SBUF
            qT_sb = [
                qkvpool.tile([128, Q], bf16, tag=f"qT{i}", name=f"qT{i}")
                for i in range(2)
            ]
            kT_sb = [
                qkvpool.tile([128, KS], bf16, tag=f"kT{i}", name=f"kT{i}")
                for i in range(2)
            ]
            v_sb = [
                qkvpool.tile([128, HPC * VW], bf16, tag=f"v{i}", name=f"v{i}")
                for i in range(KT)
            ]
            if skip_qk_bias:
                # ones columns filled once at boot; v copies never touch them
                for it in range(KT):
                    nc.gpsimd.memset(v_sb[it][:, DV::VW], 1.0)

            # ---------------- SBUF destination tiles ----------------
            w_sb = {}
            xq_g = [
                xpool.tile([128, NQK * QB], bf16, tag=f"xq{i}", name=f"xq{i}")
                for i in range(NQB)
            ]
            bq_g = [
                xpool.tile([128, NQK * QB], bf16, tag=f"bq{i}", name=f"bq{i}")
                for i in range(NQB)
            ]
            mq_g = [
                mpool.tile([128, KT * QB], bf16, tag=f"mq{i}", name=f"mq{i}")
                for i in range(NQB)
            ]

            def w_dma(nm, dram, width):
                t = wpool.tile([128, NDC * width], bf16, tag=nm, name=nm)
                src = dram[:].rearrange("(c p) w -> p c w", p=128)
                nc.sync.dma_start(
                    out=t[:].rearrange("p (c w) -> p c w", c=NDC), in_=src
                )
                w_sb[nm] = t

            def wslice(nm, dc, lo, size):
                width = (HPC * DK) if nm in ("wq", "wk") else (HPC * VW)
                return w_sb[nm][:, ds(dc * width + lo, size)]

            def xsl(dc, qc, lo=0, size=QB):
                return xq_g[qc][:, ds(dc * QB + lo, size)]

            def bsl(dc, kq, lo=0, size=QB):
                return bq_g[kq][:, ds(dc * QB + lo, size)]

            def mslice(kt, qb):
                return mq_g[qb][:, ds(kt * QB, QB)]

            # ---------------- DMAs in priority order ----------------
            w_dma("wk", wk, HPC * DK)
            nc.sync.dma_start(out=bq_g[0][:], in_=bq_d[0, :, 0 : NQK * QB])
            w_dma("wv", wv, HPC * VW)
            w_dma("wq", wq, HPC * DK)
            # mask group 0 of quarter 0 (k-tiles 0..3)
            nc.sync.dma_start(
                out=mq_g[0][:, 0 : 4 * QB], in_=mq_d[0, :, 0 : 4 * QB]
            )
            nc.sync.dma_start(out=xq_g[0][:], in_=xq_d[0, :, 0 : NQK * QB])
            for grp in range(1, 4):
                nc.sync.dma_start(
                    out=bq_g[grp][:], in_=bq_d[grp, :, 0 : NQK * QB]
                )
                nc.sync.dma_start(
                    out=mq_g[0][:, ds(grp * 4 * QB, 4 * QB)],
                    in_=mq_d[0, :, ds(grp * 4 * QB, 4 * QB)],
                )
            for qb in range(1, NQB):
                nc.sync.dma_start(
                    out=xq_g[qb][:], in_=xq_d[qb, :, 0 : NQK * QB]
                )
                nc.sync.dma_start(out=mq_g[qb][:], in_=mq_d[qb])

            # ---------------- projection units ----------------
            # two scratch banks; units alternate so unit N+1's matmuls
            # overlap unit N's PSUM->SBUF copy
            pjl = tc.tile_pool(name="scr", bufs=1, space="PSUM")
            pjlpool = pjl.__enter__()
            _ucnt = [0]

            def scratch():
                tag = f"pj{_ucnt[0] % 2}"
                _ucnt[0] += 1
                return pjlpool.tile([128, QB], f32, tag=tag, name=tag)

            def warmup():
                ps = scratch()
                for _ in range(NWARM):
                    nc.tensor.matmul(
                        ps[:],
                        ident[:],
                        warm[:],
                        start=True,
                        stop=True,
                        skip_group_check=True,
                    )

            def v_unit(it):
                """v tile it (k-positions it*128..+128): 8 matmuls + copy."""
                ps = scratch()
                ops = []
                for dc in range(NQK):
                    ops.append(
                        lambda dc=dc, ps=ps: nc.tensor.matmul(
                            ps[:, : HPC * VW],
                            bsl(dc, it // 4, (it % 4) * 128, 128),
                            wslice("wv", dc, 0, HPC * VW),
                            start=(dc == 0),
                            stop=(dc == NQK - 1),
                        )
                    )

                def fin(ps=ps):
                    if skip_qk_bias:
                        nc.vector.tensor_copy(
                            v_sb[it][:]
                            .rearrange("p (h w) -> p h w", h=HPC)[:, :, 0:DV],
                            ps[:, : HPC * VW]
                            .rearrange("p (h w) -> p h w", h=HPC)[:, :, 0:DV],
                        )
                    else:
                        nc.vector.tensor_copy(v_sb[it][:], ps[:, : HPC * VW])

                ops.append(fin)
                return ops

            def q_unit(pair, qc, on_scalar=False):
                """qT_sb[pair][:, qc*512:+512]: 8 matmuls + copy."""
                ps = scratch()
                ops = []
                for dc in range(NQK):
                    ops.append(
                        lambda dc=dc, ps=ps: nc.tensor.matmul(
                            ps[:],
                            wslice("wq", dc, pair * 128, 128),
                            xsl(dc, qc),
                            start=(dc == 0),
                            stop=(dc == NQK - 1),
                        )
                    )
                dst = qT_sb[pair]
                if on_scalar:
                    ops.append(
                        lambda ps=ps: nc.scalar.copy(dst[:, ts(qc, QB)], ps[:])
                    )
                else:
                    ops.append(
                        lambda ps=ps: nc.vector.tensor_copy(
                            dst[:, ts(qc, QB)], ps[:]
                        )
                    )
                return ops

            def k_unit(pair, qc, on_scalar=False):
                """kT_sb[pair][:, qc*512:+512]: 8 matmuls + copy."""
                ps = scratch()
                ops = []
                for dc in range(NQK):
                    ops.append(
                        lambda dc=dc, ps=ps: nc.tensor.matmul(
                            ps[:],
                            wslice("wk", dc, pair * 128, 128),
                            bsl(dc, qc),
                            start=(dc == 0),
                            stop=(dc == NQK - 1),
                        )
                    )
                dst = kT_sb[pair]
                if on_scalar:
                    ops.append(
                        lambda ps=ps: nc.scalar.copy(dst[:, ts(qc, QB)], ps[:])
                    )
                else:
                    ops.append(
                        lambda ps=ps: nc.vector.tensor_copy(
                            dst[:, ts(qc, QB)], ps[:]
                        )
                    )
                return ops

            # pre-attention: warmup, then the units block 0 needs first
            warmup()
            for ops in (
                k_unit(0, 0, on_scalar=True),
                k_unit(1, 0, on_scalar=True),
                v_unit(0),
                v_unit(1),
                q_unit(0, 0, on_scalar=True),
                v_unit(2),
                v_unit(3),
            ):
                for op in ops:
                    op()

            # drip queue: (ready_iter, emit_by_iter, unit ops)
            units = []
            units.append((0, 2, k_unit(0, 1)))
            for it in range(4, 8):
                units.append((0, it - 1, v_unit(it)))
            units.append((2, 6, k_unit(0, 2)))
            for it in range(8, 12):
                units.append((3, it - 1, v_unit(it)))
            units.append((4, 10, k_unit(0, 3)))
            for it in range(12, KT):
                units.append((5, it - 1, v_unit(it)))
            units.append((7, 11, q_unit(0, 1)))
            units.append((12, 27, q_unit(0, 2)))
            units.append((17, 43, q_unit(0, 3)))
            units.append((18, 50, k_unit(1, 1)))
            units.append((19, 54, k_unit(1, 2)))
            units.append((20, 58, k_unit(1, 3)))
            units.append((21, 56, q_unit(1, 0)))
            units.append((22, 71, q_unit(1, 1)))
            units.append((23, 87, q_unit(1, 2)))
            units.append((24, 103, q_unit(1, 3)))
            units.sort(key=lambda u: u[1])
            wq_flat = []
            for rdy, eb, ops in units:
                for op in ops:
                    wq_flat.append((rdy, eb, op))
            wq_i = 0

            def drip(giter):
                nonlocal wq_i
                n = 0
                while wq_i < len(wq_flat):
                    rdy, eb, op = wq_flat[wq_i]
                    if eb > giter and (n >= 3 or rdy > giter):
                        break
                    op()
                    wq_i += 1
                    n += 1

            # ---------------- attention ----------------
            with (
                tc.tile_pool(name="stp", bufs=2, space="PSUM") as stp,
                tc.tile_pool(name="otp", bufs=1, space="PSUM") as otp,
                tc.tile_pool(name="ptp", bufs=8) as ptp,
                tc.tile_pool(name="epp", bufs=2) as epp,
                tc.tile_pool(name="osp", bufs=3) as osp,
            ):

                def do_st(hp, qb, kt):
                    st = stp.tile([128, 2 * QB], f32, tag="st", name="st")
                    nc.tensor.matmul(
                        st[:, 0:QB],
                        kT_sb[hp][0:64, ts(kt, 128)],
                        qT_sb[hp][0:64, ds(qb * QB, QB)],
                        start=True,
                        stop=True,
                    )
                    nc.tensor.matmul(
                        st[:, QB : 2 * QB],
                        kT_sb[hp][64:128, ts(kt, 128)],
                        qT_sb[hp][64:128, ds(qb * QB, QB)],
                        start=True,
                        stop=True,
                    )
                    return st

                blocks = [(hp, qb) for hp in range(2) for qb in range(NQB)]
                st_cur = do_st(blocks[0][0], blocks[0][1], 0)
                for bi, (hp, qb) in enumerate(blocks):
                    qlo = qb * QB
                    ot0 = otp.tile([128, QB], f32, tag="ot0", name="ot0")
                    ot1 = otp.tile([128, QB], f32, tag="ot1", name="ot1")
                    for kt in range(KT):
                        giter = bi * KT + kt
                        if kt + 1 < KT:
                            st_next = do_st(hp, qb, kt + 1)
                        elif bi + 1 < len(blocks):
                            nhp, nqb = blocks[bi + 1]
                            st_next = do_st(nhp, nqb, 0)
                        else:
                            st_next = None
                        pt = ptp.tile([128, 2 * QB], bf16, tag="pt", name="pt")
                        nc.scalar.activation(pt[:], st_cur[:], EXP, scale=SCALE)
                        msl = mslice(kt, qb)
                        mbc = bass.AP(
                            tensor=msl.tensor,
                            offset=msl.offset,
                            ap=[msl.ap[0], [0, 2], [1, QB]],
                        )
                        nc.vector.tensor_mul(pt[:], pt[:], mbc)
                        nc.tensor.matmul(
                            ot0[:VW, :],
                            v_sb[kt][:, ds((2 * hp) * VW, VW)],
                            pt[:, 0:QB],
                            start=(kt == 0),
                            stop=(kt == KT - 1),
                        )
                        nc.tensor.matmul(
                            ot1[:VW, :],
                            v_sb[kt][:, ds((2 * hp + 1) * VW, VW)],
                            pt[:, QB : 2 * QB],
                            start=(kt == 0),
                            stop=(kt == KT - 1),
                        )
                        drip(giter + 1)
                        st_cur = st_next
                    # epilogue for the two heads of this (hp, qb)
                    last_block = bi == len(blocks) - 1
                    for hh, ot_acc in ((2 * hp, ot0), (2 * hp + 1, ot1)):
                        ot_sbuf = epp.tile(
                            [128, QB], bf16, tag="otsb", name="otsb"
                        )
                        nc.vector.tensor_copy(ot_sbuf[:VW, :], ot_acc[:VW, :])
                        nqt = QB // 128
                        VWP = VW + 1  # pad stride so PSUM stays 4B-aligned
                        # transpose target borrows one scratch bank
                        tr = pjlpool.tile(
                            [128, nqt * VWP], bf16, tag="pj1", name="tr"
                        )
                        for qt in range(nqt):
                            nc.tensor.transpose(
                                tr[:, ds(qt * VWP, VW)],
                                ot_sbuf[:VW, ts(qt, 128)],
                                ident[:VW, :VW],
                            )
                        rec = epp.tile([128, nqt], f32, tag="rec", name="rec")
                        nc.vector.reciprocal(rec[:], tr[:, DV::VWP])
                        osb = osp.tile(
                            [128, nqt * DV], f32, tag="osb", name="osb"
                        )
                        for qt in range(nqt):
                            if last_block:
                                nc.scalar.activation(
                                    osb[:, ds(qt * DV, DV)],
                                    tr[:, ds(qt * VWP, DV)],
                                    mybir.ActivationFunctionType.Copy,
                                    scale=rec[:, qt : qt + 1],
                                )
                            else:
                                nc.vector.tensor_scalar_mul(
                                    osb[:, ds(qt * DV, DV)],
                                    tr[:, ds(qt * VWP, DV)],
                                    rec[:, qt : qt + 1],
                                )
                        dst = out[ds(qlo, QB), ds(hh * DV, DV)].rearrange(
                            "(t p) d -> p t d", p=128
                        )
                        nc.sync.dma_start(
                            out=dst,
                            in_=osb[:].rearrange("p (t d) -> p t d", t=nqt),
                        )
            pjl.__exit__(None, None, None)
    nc.compile()
    return nc


def _get_graph(skip_qk_bias=False):
    global _GRAPH
    if _GRAPH is None or _GRAPH[1] != skip_qk_bias:
        _GRAPH = (_build_graph(skip_qk_bias), skip_qk_bias)
    return _GRAPH[0]


def _pack_quarters(aT):
    """[DPAD, N] -> [NQB, 128, NDC*512]: quarter-major, contiguous DMAs."""
    return np.ascontiguousarray(
        aT.reshape(NDC, 128, NQB, QB)
        .transpose(2, 1, 0, 3)
        .reshape(NQB, 128, NDC * QB)
    )


def _prep_core_inputs(c, x, buffer, mask, Wq, bq, Wk, bk, Wv, bv):
    bf = ml_dtypes.bfloat16
    b, g = divmod(c, 4)
    hs = slice(g * HPC * DK, (g + 1) * HPC * DK)

    xTa = np.zeros((DPAD, Q), np.float32)
    xTa[:D] = x[b].T
    xTa[D] = 1.0
    bTa = np.zeros((DPAD, KS), np.float32)
    bTa[:D] = buffer[b].T
    bTa[D] = 1.0
    wqa = np.zeros((DPAD, HPC * DK), np.float32)
    wqa[:D] = Wq[hs].T
    wqa[D] = bq[hs]
    wka = np.zeros((DPAD, HPC * DK), np.float32)
    wka[:D] = Wk[hs].T
    wka[D] = bk[hs]
    wva = np.zeros((DPAD, HPC * VW), np.float32)
    for hh in range(HPC):
        gh = g * HPC + hh
        wva[:D, hh * VW : hh * VW + DV] = Wv[gh * DV : (gh + 1) * DV].T
        wva[D, hh * VW : hh * VW + DV] = bv[gh * DV : (gh + 1) * DV]
        wva[D, hh * VW + DV] = 1.0
    mTa = mask[b].T.astype(np.float32)  # [KS, Q]
    # mask quarters: mq[qb][p][kt*QB + w] = mTa[kt*128 + p][qb*QB + w]
    mqa = (
        mTa.reshape(KT, 128, NQB, QB)
        .transpose(2, 1, 0, 3)
        .reshape(NQB, 128, KT * QB)
    )
    return {
        "xq": _pack_quarters(xTa).astype(bf),
        "bq": _pack_quarters(bTa).astype(bf),
        "wq": wqa.astype(bf),
        "wk": wka.astype(bf),
        "wv": wva.astype(bf),
        "mq": np.ascontiguousarray(mqa).astype(bf),
    }


def kernel(**inputs):
    x = np.asarray(inputs["x"], dtype=np.float32)
    buffer = np.asarray(inputs["buffer"], dtype=np.float32)
    mask = np.asarray(inputs["mask"])
    Wq = np.asarray(inputs["Wq"], dtype=np.float32)
    bq = np.asarray(inputs["bq"], dtype=np.float32)
    Wk = np.asarray(inputs["Wk"], dtype=np.float32)
    bk = np.asarray(inputs["bk"], dtype=np.float32)
    Wv = np.asarray(inputs["Wv"], dtype=np.float32)
    bv = np.asarray(inputs["bv"], dtype=np.float32)

    from concourse.bass_utils import run_bass_kernel_spmd

    skip_qk_bias = not (bq.any() or bk.any())
    nc = _get_graph(skip_qk_bias)
    in_maps = [
        _prep_core_inputs(c, x, buffer, mask, Wq, bq, Wk, bk, Wv, bv)
        for c in range(NCORES)
    ]
    res = run_bass_kernel_spmd(nc, in_maps, core_ids=list(range(NCORES)))
    full = np.empty((B, Q, H * DV), np.float32)
    for c in range(NCORES):
        b, g = divmod(c, 4)
        full[b, :, g * HPC * DV : (g + 1) * HPC * DV] = res.results[c]["out"]
    return full
